# revision 13
# baseline (speedup 1.0000x reference)
"""AttentionPooling (segment softmax-pool) Trainium2 kernel.

Math (per reference):
    h      = gelu(x @ W1 + b1)            # [N, H]
    s      = h @ W2 + b2                  # [N]
    w      = softmax_per_segment(s)       # segments from sorted `batch`
    pooled = segment_sum(w[:, None] * x)  # [B, D]

Strategy (8 NeuronCores, data-parallel over N):
  - Shard rows across 8 cores. Each core streams its rows once in bf16
    natural layout (xap, for the pooling matmul) and once in fp8e4
    transposed layout (xht8, for the score MLP only — fp8 score error is
    ~1e-2 relative on softmax weights, which the pooled average tolerates),
    in groups of KST macro-tiles (one macro = 512 rows):
      * scores via the tiny MLP on the tensor engine (fp8 in, f32 psum)
      * e = exp(s + b2) computed WITHOUT the Exp activation table (avoids
        Gelu<->Exp table thrash): t = tanh((s+b2)/2) on ACT (tanh lives in
        the gelu table set), then e = (1+t)/(1-t) on DVE with a fast
        reciprocal custom op; evaluated per half-group so the A matrix for
        the group is ready well before the next group's pooling needs it
      * a one-hot-times-e matrix A[row, seg-in-window] built with
        iota/is_equal on the vector engine (window = [b_lo_m, b_lo_m + W)),
        one instruction pair per half-group
      * windowed pooled partials P_m[W, D] = sum_rows e_i * x_i via matmul
        (x stationary, A moving), f32 PSUM accumulation; a whole group's
        macros accumulate into one PSUM bank tile, one DVE copy per group
  - Device ships per-macro windows P_m and per-row e back to HBM.
  - Host scatter-adds the windows at their (host-known) b_lo_m offsets,
    computes denominators from e, combines the 8 cores, and divides.
    Softmax max-subtraction is skipped: scores are O(1) for this model, and
    softmax is shift-invariant, so exp() cannot overflow.
"""

import sys

import numpy as np

sys.path.insert(0, "/opt/trn_rl_repo")

import ml_dtypes

N_CORES = 8
D = 128  # feature dim
H = 128  # hidden dim
NSEG = 1024
PAD_SEG = NSEG  # extra segment id for padding rows
CHUNK = 128  # rows per PE contraction
CH = 4  # chunks per macro
MACRO = CHUNK * CH  # 512 rows
KST = 32  # macros per group (DMA/activation batch)
HK = KST // 2  # macros per half-group (e/A-matrix batch)

_prog_cache: dict = {}


def _build_program(NM: int, W: int, act_name: str = "Gelu"):
    """Emit + compile the per-core Tile program. NM macros per core (multiple
    of KST), segment window W."""
    from contextlib import ExitStack

    import concourse.tile as tile
    from concourse import bacc, mybir
    from concourse.dve_ops import RECIP_APPROX_FAST_CONSTS, RECIPROCAL_APPROX_FAST

    bf16 = mybir.dt.bfloat16
    f32 = mybir.dt.float32
    fp8 = mybir.dt.float8e4
    AF = mybir.ActivationFunctionType
    ALU = mybir.AluOpType

    assert NM % KST == 0
    NG = NM // KST
    Nc = NM * MACRO
    GROWS = KST * MACRO  # rows per group

    nc = bacc.Bacc("TRN2", target_bir_lowering=False, debug=False, num_devices=N_CORES)

    xap = nc.dram_tensor("xap", [CHUNK, NM, CH, D], bf16, kind="ExternalInput")
    xht8 = nc.dram_tensor("xht8", [D, Nc], fp8, kind="ExternalInput")
    brel = nc.dram_tensor("brel", [128, NM, CH], bf16, kind="ExternalInput")
    w1 = nc.dram_tensor("w1", [D, H], fp8, kind="ExternalInput")
    w2 = nc.dram_tensor("w2", [H, 1], bf16, kind="ExternalInput")
    b1 = nc.dram_tensor("b1", [H, 1], f32, kind="ExternalInput")
    b2h = nc.dram_tensor("b2h", [128, 1], f32, kind="ExternalInput")
    iota = nc.dram_tensor("iota", [128, W], bf16, kind="ExternalInput")
    pool_out = nc.dram_tensor("pool_out", [D, NM, W], f32, kind="ExternalOutput")
    e_out = nc.dram_tensor("e_out", [128, NM, CH], bf16, kind="ExternalOutput")

    # DRAM views (all host-prepped layouts are contiguous per partition)
    xa_view = xap.ap().rearrange("p (g k) j d -> g p k j d", k=KST)
    xt_view = xht8.ap().rearrange("d (g n) -> g d n", n=GROWS)

    with tile.TileContext(nc) as tc, ExitStack() as ctx:
        pool = lambda name, bufs, **kw: ctx.enter_context(
            tc.tile_pool(name=name, bufs=bufs, **kw)
        )
        p_const = pool("const", 1)
        p_xa = pool("xa", 3)
        p_xt = pool("xt", 3)
        p_bt = pool("bt", 3)
        p_hg = pool("hg", 3)
        p_a = pool("amat", 2)
        p_ts = pool("tstage", 2)
        p_es = pool("estage", 2)
        p_ps = pool("pstage", 2)
        p_hp = pool("hpsum", 3, space="PSUM")
        p_sc = pool("scpsum", 1, space="PSUM")
        p_pp = pool("ppsum", 1, space="PSUM")

        w1_sb = p_const.tile([D, H], fp8)
        nc.sync.dma_start(w1_sb[:], w1.ap())
        w2_sb = p_const.tile([H, 1], bf16)
        nc.sync.dma_start(w2_sb[:], w2.ap())
        b1_sb = p_const.tile([H, 1], f32)
        nc.sync.dma_start(b1_sb[:], b1.ap())
        b2h_sb = p_const.tile([128, 1], f32)
        nc.sync.dma_start(b2h_sb[:], b2h.ap())
        iota_sb = p_const.tile([128, W], bf16)
        nc.sync.dma_start(iota_sb[:], iota.ap())

        def emit_echain(sc_half, bt, m0, h):
            """e = (1+tanh((s+b2)/2)) / (1-tanh(..)) for macros
            [h*HK, (h+1)*HK); build this half's A matrix."""
            tst = p_ts.tile([128, HK, CH], f32)
            nc.scalar.activation(
                tst[:].rearrange("p k j -> p (k j)"),
                sc_half.rearrange("p k j -> p (k j)"),
                AF.Tanh,
                bias=b2h_sb[:],
                scale=0.5,
            )
            den = p_ts.tile([128, HK, CH], f32)
            nc.vector.tensor_scalar(den[:], tst[:], -1.0, 1.0, ALU.mult, ALU.add)
            rec = p_ts.tile([128, HK, CH], f32)
            nc.vector._custom_dve(
                RECIPROCAL_APPROX_FAST, out=rec[:], in0=den[:],
                **RECIP_APPROX_FAST_CONSTS,
            )
            num = p_ts.tile([128, HK, CH], f32)
            nc.vector.tensor_scalar(num[:], tst[:], 1.0, None, ALU.add)
            estage = p_es.tile([128, HK, CH], bf16)
            nc.vector.tensor_tensor(estage[:], num[:], rec[:], ALU.mult)
            nc.gpsimd.dma_start(
                e_out.ap()[:, m0 + h * HK : m0 + (h + 1) * HK, :], estage[:]
            )
            amat = p_a.tile([128, HK, CH, W], bf16)
            nc.vector.tensor_tensor(
                out=amat[:],
                in0=iota_sb[:].unsqueeze(1).unsqueeze(1).broadcast_to(
                    [128, HK, CH, W]
                ),
                in1=bt[:, h * HK : (h + 1) * HK, :].unsqueeze(3).broadcast_to(
                    [128, HK, CH, W]
                ),
                op=ALU.is_equal,
            )
            nc.vector.tensor_tensor(
                out=amat[:],
                in0=amat[:],
                in1=estage[:].unsqueeze(3).broadcast_to([128, HK, CH, W]),
                op=ALU.mult,
            )
            return amat

        # Software pipeline: iteration g runs the scores pass for group g
        # interleaved (macro-by-macro, so PE/ACT/DVE all stay busy) with the
        # pooling pass for group g-1.
        prev = None  # (xa, [amat_half0, amat_half1], m0) of group g-1
        for g in range(NG + 1):
            if g < NG:
                m0 = g * KST
                # first groups via the HWDGE sync queue: the SWDGE (gpsimd)
                # queue starts late (library load) and would stall the ramp.
                # xt before xa: the first scores only need xt (1MB vs 4MB).
                q = nc.sync if g < 2 else nc.gpsimd
                xt = p_xt.tile([128, KST, MACRO], fp8)
                q.dma_start(xt[:], xt_view[g])
                bt = p_bt.tile([128, KST, CH], bf16)
                q.dma_start(bt[:], brel.ap()[:, m0 : m0 + KST, :])
                # alternate the big bf16 stream between the two HWDGE queues
                # (sync and scalar) so neither falls behind; xa[g] is first
                # consumed one group later, hiding the scalar queue's issue lag
                xa = p_xa.tile([128, KST, CH, CHUNK], bf16)
                (nc.sync if g % 2 == 0 else nc.scalar).dma_start(xa[:], xa_view[g])
                sc_g = p_sc.tile([128, 2, HK, CH], f32, space="PSUM")
                amats = []

            if prev is not None:
                pstage = p_ps.tile([D, KST, W], f32)
                pp = p_pp.tile([128, KST, W], f32, space="PSUM")

            def emit_mm2(hg, i, k):
                h, k_ = (0, k) if k < HK else (1, k - HK)
                for j in range(CH):
                    nc.tensor.matmul(
                        sc_g[:, h, k_, j : j + 1],
                        lhsT=hg[:, i, j * CHUNK : (j + 1) * CHUNK],
                        rhs=w2_sb[:],
                        start=True,
                        stop=True,
                    )

            def emit_pool(k):
                pxa, pams, _ = prev
                pam = pams[0] if k < HK else pams[1]
                k_ = k if k < HK else k - HK
                for j in range(CH):
                    nc.tensor.matmul(
                        pp[:, k, :], lhsT=pxa[:, k, j, :], rhs=pam[:, k_, j, :],
                        start=(j == 0), stop=(j == CH - 1),
                    )

            # Macro pairs: mm1(k), mm1(k+1) into a 2-bank psum tile, one gelu
            # over both; pooling matmuls of (g-1) interleave to cover the
            # gelu latency in PE program order, and each pair's mm2s are
            # delayed one pair so they never wait on their gelu.
            pend_mm2 = None
            for kk in range(0, KST, 2):
                if g < NG:
                    hp = p_hp.tile([128, 2, MACRO], f32, space="PSUM")
                    nc.tensor.matmul(
                        hp[:, 0, :], lhsT=w1_sb[:], rhs=xt[:, kk, :],
                        start=True, stop=True,
                    )
                    nc.tensor.matmul(
                        hp[:, 1, :], lhsT=w1_sb[:], rhs=xt[:, kk + 1, :],
                        start=True, stop=True,
                    )
                    hg = p_hg.tile([128, 2, MACRO], bf16)
                    nc.scalar.activation(
                        hg[:].rearrange("p i r -> p (i r)"),
                        hp[:].rearrange("p i r -> p (i r)"),
                        getattr(AF, act_name),
                        bias=b1_sb[:],
                        scale=1.0,
                    )
                if pend_mm2 is not None:
                    emit_mm2(pend_mm2, 0, kk - 2)
                    emit_mm2(pend_mm2, 1, kk - 1)
                if prev is not None:
                    emit_pool(kk)
                    emit_pool(kk + 1)
                if g < NG:
                    pend_mm2 = hg
                    if kk == HK:
                        # macros [0, HK) have their mm2s done (pair delay=1):
                        # run the first half's e/A chain now so the next
                        # group's pooling never waits on it
                        amats.append(emit_echain(sc_g[:, 0], bt, m0, 0))
            if pend_mm2 is not None:
                emit_mm2(pend_mm2, 0, KST - 2)
                emit_mm2(pend_mm2, 1, KST - 1)

            if prev is not None:
                # flush group g-1 on the (otherwise idle) gpsimd SWDGE queue
                pm0 = prev[2]
                nc.vector.tensor_copy(pstage[:], pp[:])
                nc.gpsimd.dma_start(pool_out.ap()[:, pm0 : pm0 + KST, :], pstage[:])

            if g < NG:
                amats.append(emit_echain(sc_g[:, 1], bt, m0, 1))
                prev = (xa, amats, m0)

    nc.compile()
    return nc


def _prep_inputs(x, batch, W1, b1, W2, b2):
    """Host-side shard + preprocess. Returns (in_maps, meta)."""
    bf = ml_dtypes.bfloat16
    f8 = ml_dtypes.float8_e4m3
    x = np.asarray(x)
    batch = np.asarray(batch)
    N = x.shape[0]

    NM = -(-N // (N_CORES * MACRO))  # macros per core
    NM = -(-NM // KST) * KST  # round up to full groups
    NP = N_CORES * NM * MACRO
    Nc = NM * MACRO

    xhi = np.zeros((NP, D), dtype=bf)
    xhi[:N] = x.astype(bf)
    x8 = np.zeros((NP, D), dtype=f8)
    x8[:N] = x.astype(f8)
    bpad = np.full(NP, PAD_SEG, dtype=np.int64)
    bpad[:N] = batch

    bv = bpad.reshape(N_CORES, NM, MACRO)
    # window start per macro; pad id is the largest so min() tracks real rows
    blo = bv.min(axis=2)  # [8, NM]
    # window width from real rows only
    real = bv != PAD_SEG
    breal_max = np.where(real, bv, -1).max(axis=2)  # -1 if all pad
    span = np.maximum(breal_max - blo + 1, 1)
    W = int(max(8, span.max()))
    assert W <= 128, f"segment window {W} too wide"

    brel = (bv - blo[:, :, None]).astype(np.float32)  # [8, NM, 512]
    # device layout: brel_dev[c, p, m, j] = brel[c, m, j*128 + p]
    brel_dev = np.ascontiguousarray(
        brel.reshape(N_CORES, NM, CH, CHUNK).transpose(0, 3, 1, 2).astype(bf)
    )

    iota_arr = np.ascontiguousarray(
        np.broadcast_to(np.arange(W, dtype=np.float32).astype(bf), (128, W))
    )
    w1c = np.ascontiguousarray(np.asarray(W1).astype(f8))
    w2c = np.ascontiguousarray(np.asarray(W2).astype(bf))
    b1c = np.ascontiguousarray(np.asarray(b1, dtype=np.float32).reshape(H, 1))
    b2h = np.full(
        (128, 1), 0.5 * np.asarray(b2, dtype=np.float32).ravel()[0], np.float32
    )

    in_maps = []
    for c in range(N_CORES):
        xc = xhi[c * Nc : (c + 1) * Nc]
        x8c = x8[c * Nc : (c + 1) * Nc]
        in_maps.append(
            {
                # xap[p, m, j, :] = x[m*512 + j*128 + p, :]
                "xap": np.ascontiguousarray(
                    xc.reshape(NM, CH, CHUNK, D).transpose(2, 0, 1, 3)
                ),
                "xht8": np.ascontiguousarray(x8c.T),
                "brel": brel_dev[c],
                "w1": w1c,
                "w2": w2c,
                "b1": b1c,
                "b2h": b2h,
                "iota": iota_arr,
            }
        )
    meta = {"NM": NM, "W": W, "Nc": Nc, "NP": NP, "N": N, "blo": blo, "bpad": bpad}
    return in_maps, meta


def _combine(results, meta):
    """Host unshard: scatter-add macro windows, divide by segment denominators."""
    NM, W, Nc = meta["NM"], meta["W"], meta["Nc"]
    blo, bpad = meta["blo"], meta["bpad"]

    seg_acc = np.zeros((NSEG + 1, D), dtype=np.float64)
    e_all = np.empty(N_CORES * Nc, dtype=np.float32)
    wofs = np.arange(W)
    for c in range(N_CORES):
        po = np.asarray(results[c]["pool_out"], dtype=np.float64)  # [D, NM, W]
        seg_idx = (blo[c][:, None] + wofs[None, :]).ravel()  # [NM*W]
        valid = seg_idx <= NSEG
        contrib = po.transpose(1, 2, 0).reshape(-1, D)  # [NM*W, D]
        np.add.at(seg_acc, seg_idx[valid], contrib[valid])
        # e_dev[p, m, j] -> row m*512 + j*128 + p
        e_dev = np.asarray(results[c]["e_out"]).astype(np.float32)  # [128, NM, CH]
        e_all[c * Nc : (c + 1) * Nc] = e_dev.transpose(1, 2, 0).reshape(Nc)

    denom = np.bincount(bpad, weights=e_all.astype(np.float64), minlength=NSEG + 1)
    denom = denom[:NSEG]
    out = seg_acc[:NSEG]
    safe = denom != 0
    pooled = np.zeros((NSEG, D), dtype=np.float32)
    pooled[safe] = (out[safe] / denom[safe, None]).astype(np.float32)
    return pooled


def _run(inputs: dict, trace: bool = False):
    from concourse.bass_utils import run_bass_kernel_spmd

    in_maps, meta = _prep_inputs(
        inputs["x"], inputs["batch"], inputs["W1"], inputs["b1"], inputs["W2"],
        inputs["b2"],
    )
    key = (meta["NM"], meta["W"])
    if key not in _prog_cache:
        _prog_cache[key] = _build_program(*key)
    nc = _prog_cache[key]
    res = run_bass_kernel_spmd(
        nc, in_maps, core_ids=list(range(N_CORES)), trace=trace
    )
    pooled = _combine(res.results, meta)
    return pooled, res


def kernel(**inputs) -> np.ndarray:
    pooled, _ = _run(inputs, trace=False)
    return pooled


# revision 14
# speedup vs baseline: 1.2493x; 1.2493x over previous
"""AttentionPooling (segment softmax-pool) Trainium2 kernel.

Math (per reference):
    h      = gelu(x @ W1 + b1)            # [N, H]
    s      = h @ W2 + b2                  # [N]
    w      = softmax_per_segment(s)       # segments from sorted `batch`
    pooled = segment_sum(w[:, None] * x)  # [B, D]

Strategy (8 NeuronCores, data-parallel over N):
  - Shard rows across 8 cores. Each core streams its rows once in bf16
    natural layout (xap, for the pooling matmul) and once in fp8e4
    transposed layout (xht8, for the score MLP only — fp8 score error is
    ~1e-2 relative on softmax weights, which the pooled average tolerates),
    in groups of KST macro-tiles (one macro = 512 rows):
      * scores via the tiny MLP on the tensor engine (fp8 in, f32 psum)
      * e = exp(s + b2) computed WITHOUT the Exp activation table (avoids
        Gelu<->Exp table thrash): t = tanh((s+b2)/2) on ACT (tanh lives in
        the gelu table set), then e = (1+t)/(1-t) on DVE with a fast
        reciprocal custom op; evaluated per half-group so the A matrix for
        the group is ready well before the next group's pooling needs it
      * a one-hot-times-e matrix A[row, seg-in-window] built with
        iota/is_equal on the vector engine (window = [b_lo_m, b_lo_m + W)),
        one instruction pair per half-group
      * windowed pooled partials P_m[W, D] = sum_rows e_i * x_i via matmul
        (x stationary, A moving), f32 PSUM accumulation; a whole group's
        macros accumulate into one PSUM bank tile, one DVE copy per group
  - Device ships per-macro windows P_m and per-row e back to HBM.
  - Host scatter-adds the windows at their (host-known) b_lo_m offsets,
    computes denominators from e, combines the 8 cores, and divides.
    Softmax max-subtraction is skipped: scores are O(1) for this model, and
    softmax is shift-invariant, so exp() cannot overflow.
"""

import sys

import numpy as np

sys.path.insert(0, "/opt/trn_rl_repo")

import ml_dtypes

N_CORES = 8
D = 128  # feature dim
H = 128  # hidden dim
NSEG = 1024
PAD_SEG = NSEG  # extra segment id for padding rows
CHUNK = 128  # rows per PE contraction
CH = 4  # chunks per macro
MACRO = CHUNK * CH  # 512 rows
KST = 32  # macros per group (DMA/activation batch)
HK = KST // 2  # macros per half-group (e/A-matrix batch)

_prog_cache: dict = {}


def _build_program(NM: int, W: int, act_name: str = "Gelu"):
    """Emit + compile the per-core Tile program. NM macros per core (multiple
    of KST), segment window W."""
    from contextlib import ExitStack

    import concourse.tile as tile
    from concourse import bacc, mybir
    from concourse.dve_ops import RECIP_APPROX_FAST_CONSTS, RECIPROCAL_APPROX_FAST

    bf16 = mybir.dt.bfloat16
    f32 = mybir.dt.float32
    fp8 = mybir.dt.float8e4
    AF = mybir.ActivationFunctionType
    ALU = mybir.AluOpType

    assert NM % KST == 0
    NG = NM // KST
    Nc = NM * MACRO
    GROWS = KST * MACRO  # rows per group

    nc = bacc.Bacc("TRN2", target_bir_lowering=False, debug=False, num_devices=N_CORES)

    xap = nc.dram_tensor("xap", [CHUNK, NM, CH, D], bf16, kind="ExternalInput")
    xht8 = nc.dram_tensor("xht8", [D, Nc], fp8, kind="ExternalInput")
    brel = nc.dram_tensor("brel", [128, NM, CH], bf16, kind="ExternalInput")
    w1 = nc.dram_tensor("w1", [D, H], fp8, kind="ExternalInput")
    w2 = nc.dram_tensor("w2", [H, 1], bf16, kind="ExternalInput")
    b1 = nc.dram_tensor("b1", [H, 1], f32, kind="ExternalInput")
    b2h = nc.dram_tensor("b2h", [128, 1], f32, kind="ExternalInput")
    iota = nc.dram_tensor("iota", [128, W], bf16, kind="ExternalInput")
    pool_out = nc.dram_tensor("pool_out", [D, NM, W], f32, kind="ExternalOutput")
    e_out = nc.dram_tensor("e_out", [128, NM, CH], bf16, kind="ExternalOutput")

    # DRAM views (all host-prepped layouts are contiguous per partition)
    xa_view = xap.ap().rearrange("p (g k) j d -> g p k j d", k=KST)
    xt_view = xht8.ap().rearrange("d (g n) -> g d n", n=GROWS)

    with tile.TileContext(nc) as tc, ExitStack() as ctx:
        pool = lambda name, bufs, **kw: ctx.enter_context(
            tc.tile_pool(name=name, bufs=bufs, **kw)
        )
        p_const = pool("const", 1)
        p_xa = pool("xa", 3)
        p_xt = pool("xt", 3)
        p_bt = pool("bt", 3)
        p_hg = pool("hg", 3)
        p_a = pool("amat", 2)
        p_ts = pool("tstage", 2)
        p_es = pool("estage", 2)
        p_ps = pool("pstage", 2)
        p_hp = pool("hpsum", 3, space="PSUM")
        p_sc = pool("scpsum", 1, space="PSUM")
        p_pp = pool("ppsum", 1, space="PSUM")

        w1_sb = p_const.tile([D, H], fp8)
        nc.sync.dma_start(w1_sb[:], w1.ap())
        w2_sb = p_const.tile([H, 1], bf16)
        nc.sync.dma_start(w2_sb[:], w2.ap())
        b1_sb = p_const.tile([H, 1], f32)
        nc.sync.dma_start(b1_sb[:], b1.ap())
        b2h_sb = p_const.tile([128, 1], f32)
        nc.sync.dma_start(b2h_sb[:], b2h.ap())
        iota_sb = p_const.tile([128, W], bf16)
        nc.sync.dma_start(iota_sb[:], iota.ap())

        def emit_echain(sc_half, bt, m0, h):
            """e = (1+tanh((s+b2)/2)) / (1-tanh(..)) for macros
            [h*HK, (h+1)*HK); build this half's A matrix."""
            tst = p_ts.tile([128, HK, CH], f32)
            nc.scalar.activation(
                tst[:].rearrange("p k j -> p (k j)"),
                sc_half.rearrange("p k j -> p (k j)"),
                AF.Tanh,
                bias=b2h_sb[:],
                scale=0.5,
            )
            den = p_ts.tile([128, HK, CH], f32)
            nc.vector.tensor_scalar(den[:], tst[:], -1.0, 1.0, ALU.mult, ALU.add)
            rec = p_ts.tile([128, HK, CH], f32)
            nc.vector._custom_dve(
                RECIPROCAL_APPROX_FAST, out=rec[:], in0=den[:],
                **RECIP_APPROX_FAST_CONSTS,
            )
            num = p_ts.tile([128, HK, CH], f32)
            nc.vector.tensor_scalar(num[:], tst[:], 1.0, None, ALU.add)
            estage = p_es.tile([128, HK, CH], bf16)
            nc.vector.tensor_tensor(estage[:], num[:], rec[:], ALU.mult)
            nc.gpsimd.dma_start(
                e_out.ap()[:, m0 + h * HK : m0 + (h + 1) * HK, :], estage[:]
            )
            amat = p_a.tile([128, HK, CH, W], bf16)
            nc.vector.tensor_tensor(
                out=amat[:],
                in0=iota_sb[:].unsqueeze(1).unsqueeze(1).broadcast_to(
                    [128, HK, CH, W]
                ),
                in1=bt[:, h * HK : (h + 1) * HK, :].unsqueeze(3).broadcast_to(
                    [128, HK, CH, W]
                ),
                op=ALU.is_equal,
            )
            nc.vector.tensor_tensor(
                out=amat[:],
                in0=amat[:],
                in1=estage[:].unsqueeze(3).broadcast_to([128, HK, CH, W]),
                op=ALU.mult,
            )
            return amat

        # Software pipeline: iteration g runs the scores pass for group g
        # interleaved (macro-by-macro, so PE/ACT/DVE all stay busy) with the
        # pooling pass for group g-1.
        prev = None  # (xa, [amat_half0, amat_half1], m0) of group g-1
        for g in range(NG + 1):
            if g < NG:
                m0 = g * KST
                # first groups via the HWDGE sync queue: the SWDGE (gpsimd)
                # queue starts late (library load) and would stall the ramp.
                # xt before xa: the first scores only need xt (1MB vs 4MB).
                q = nc.sync if g < 2 else nc.gpsimd
                xt = p_xt.tile([128, KST, MACRO], fp8)
                q.dma_start(xt[:], xt_view[g])
                bt = p_bt.tile([128, KST, CH], bf16)
                q.dma_start(bt[:], brel.ap()[:, m0 : m0 + KST, :])
                xa = p_xa.tile([128, KST, CH, CHUNK], bf16)
                nc.sync.dma_start(xa[:], xa_view[g])
                sc_g = p_sc.tile([128, 2, HK, CH], f32, space="PSUM")
                amats = []

            if prev is not None:
                pstage = p_ps.tile([D, KST, W], f32)
                pp = p_pp.tile([128, KST, W], f32, space="PSUM")

            def emit_mm2(hg, i, k):
                h, k_ = (0, k) if k < HK else (1, k - HK)
                for j in range(CH):
                    nc.tensor.matmul(
                        sc_g[:, h, k_, j : j + 1],
                        lhsT=hg[:, i, j * CHUNK : (j + 1) * CHUNK],
                        rhs=w2_sb[:],
                        start=True,
                        stop=True,
                    )

            def emit_pool(k):
                pxa, pams, _ = prev
                pam = pams[0] if k < HK else pams[1]
                k_ = k if k < HK else k - HK
                for j in range(CH):
                    nc.tensor.matmul(
                        pp[:, k, :], lhsT=pxa[:, k, j, :], rhs=pam[:, k_, j, :],
                        start=(j == 0), stop=(j == CH - 1),
                    )

            # Macro pairs: mm1(k), mm1(k+1) into a 2-bank psum tile, one gelu
            # over both; pooling matmuls of (g-1) interleave to cover the
            # gelu latency in PE program order, and each pair's mm2s are
            # delayed one pair so they never wait on their gelu.
            pend_mm2 = None
            for kk in range(0, KST, 2):
                if g < NG:
                    hp = p_hp.tile([128, 2, MACRO], f32, space="PSUM")
                    nc.tensor.matmul(
                        hp[:, 0, :], lhsT=w1_sb[:], rhs=xt[:, kk, :],
                        start=True, stop=True,
                    )
                    nc.tensor.matmul(
                        hp[:, 1, :], lhsT=w1_sb[:], rhs=xt[:, kk + 1, :],
                        start=True, stop=True,
                    )
                    hg = p_hg.tile([128, 2, MACRO], bf16)
                    nc.scalar.activation(
                        hg[:].rearrange("p i r -> p (i r)"),
                        hp[:].rearrange("p i r -> p (i r)"),
                        getattr(AF, act_name),
                        bias=b1_sb[:],
                        scale=1.0,
                    )
                if pend_mm2 is not None:
                    emit_mm2(pend_mm2, 0, kk - 2)
                    emit_mm2(pend_mm2, 1, kk - 1)
                if prev is not None:
                    emit_pool(kk)
                    emit_pool(kk + 1)
                if g < NG:
                    pend_mm2 = hg
                    if kk == HK:
                        # macros [0, HK) have their mm2s done (pair delay=1):
                        # run the first half's e/A chain now so the next
                        # group's pooling never waits on it
                        amats.append(emit_echain(sc_g[:, 0], bt, m0, 0))
            if pend_mm2 is not None:
                emit_mm2(pend_mm2, 0, KST - 2)
                emit_mm2(pend_mm2, 1, KST - 1)

            if prev is not None:
                # flush group g-1 on the (otherwise idle) gpsimd SWDGE queue
                pm0 = prev[2]
                nc.vector.tensor_copy(pstage[:], pp[:])
                nc.gpsimd.dma_start(pool_out.ap()[:, pm0 : pm0 + KST, :], pstage[:])

            if g < NG:
                amats.append(emit_echain(sc_g[:, 1], bt, m0, 1))
                prev = (xa, amats, m0)

    nc.compile()
    return nc


def _prep_inputs(x, batch, W1, b1, W2, b2):
    """Host-side shard + preprocess. Returns (in_maps, meta)."""
    bf = ml_dtypes.bfloat16
    f8 = ml_dtypes.float8_e4m3
    x = np.asarray(x)
    batch = np.asarray(batch)
    N = x.shape[0]

    NM = -(-N // (N_CORES * MACRO))  # macros per core
    NM = -(-NM // KST) * KST  # round up to full groups
    NP = N_CORES * NM * MACRO
    Nc = NM * MACRO

    xhi = np.zeros((NP, D), dtype=bf)
    xhi[:N] = x.astype(bf)
    x8 = np.zeros((NP, D), dtype=f8)
    x8[:N] = x.astype(f8)
    bpad = np.full(NP, PAD_SEG, dtype=np.int64)
    bpad[:N] = batch

    bv = bpad.reshape(N_CORES, NM, MACRO)
    # window start per macro; pad id is the largest so min() tracks real rows
    blo = bv.min(axis=2)  # [8, NM]
    # window width from real rows only
    real = bv != PAD_SEG
    breal_max = np.where(real, bv, -1).max(axis=2)  # -1 if all pad
    span = np.maximum(breal_max - blo + 1, 1)
    W = int(max(8, span.max()))
    assert W <= 128, f"segment window {W} too wide"

    brel = (bv - blo[:, :, None]).astype(np.float32)  # [8, NM, 512]
    # device layout: brel_dev[c, p, m, j] = brel[c, m, j*128 + p]
    brel_dev = np.ascontiguousarray(
        brel.reshape(N_CORES, NM, CH, CHUNK).transpose(0, 3, 1, 2).astype(bf)
    )

    iota_arr = np.ascontiguousarray(
        np.broadcast_to(np.arange(W, dtype=np.float32).astype(bf), (128, W))
    )
    w1c = np.ascontiguousarray(np.asarray(W1).astype(f8))
    w2c = np.ascontiguousarray(np.asarray(W2).astype(bf))
    b1c = np.ascontiguousarray(np.asarray(b1, dtype=np.float32).reshape(H, 1))
    b2h = np.full(
        (128, 1), 0.5 * np.asarray(b2, dtype=np.float32).ravel()[0], np.float32
    )

    in_maps = []
    for c in range(N_CORES):
        xc = xhi[c * Nc : (c + 1) * Nc]
        x8c = x8[c * Nc : (c + 1) * Nc]
        in_maps.append(
            {
                # xap[p, m, j, :] = x[m*512 + j*128 + p, :]
                "xap": np.ascontiguousarray(
                    xc.reshape(NM, CH, CHUNK, D).transpose(2, 0, 1, 3)
                ),
                "xht8": np.ascontiguousarray(x8c.T),
                "brel": brel_dev[c],
                "w1": w1c,
                "w2": w2c,
                "b1": b1c,
                "b2h": b2h,
                "iota": iota_arr,
            }
        )
    meta = {"NM": NM, "W": W, "Nc": Nc, "NP": NP, "N": N, "blo": blo, "bpad": bpad}
    return in_maps, meta


def _combine(results, meta):
    """Host unshard: scatter-add macro windows, divide by segment denominators."""
    NM, W, Nc = meta["NM"], meta["W"], meta["Nc"]
    blo, bpad = meta["blo"], meta["bpad"]

    seg_acc = np.zeros((NSEG + 1, D), dtype=np.float64)
    e_all = np.empty(N_CORES * Nc, dtype=np.float32)
    wofs = np.arange(W)
    for c in range(N_CORES):
        po = np.asarray(results[c]["pool_out"], dtype=np.float64)  # [D, NM, W]
        seg_idx = (blo[c][:, None] + wofs[None, :]).ravel()  # [NM*W]
        valid = seg_idx <= NSEG
        contrib = po.transpose(1, 2, 0).reshape(-1, D)  # [NM*W, D]
        np.add.at(seg_acc, seg_idx[valid], contrib[valid])
        # e_dev[p, m, j] -> row m*512 + j*128 + p
        e_dev = np.asarray(results[c]["e_out"]).astype(np.float32)  # [128, NM, CH]
        e_all[c * Nc : (c + 1) * Nc] = e_dev.transpose(1, 2, 0).reshape(Nc)

    denom = np.bincount(bpad, weights=e_all.astype(np.float64), minlength=NSEG + 1)
    denom = denom[:NSEG]
    out = seg_acc[:NSEG]
    safe = denom != 0
    pooled = np.zeros((NSEG, D), dtype=np.float32)
    pooled[safe] = (out[safe] / denom[safe, None]).astype(np.float32)
    return pooled


def _run(inputs: dict, trace: bool = False):
    from concourse.bass_utils import run_bass_kernel_spmd

    in_maps, meta = _prep_inputs(
        inputs["x"], inputs["batch"], inputs["W1"], inputs["b1"], inputs["W2"],
        inputs["b2"],
    )
    key = (meta["NM"], meta["W"])
    if key not in _prog_cache:
        _prog_cache[key] = _build_program(*key)
    nc = _prog_cache[key]
    res = run_bass_kernel_spmd(
        nc, in_maps, core_ids=list(range(N_CORES)), trace=trace
    )
    pooled = _combine(res.results, meta)
    return pooled, res


def kernel(**inputs) -> np.ndarray:
    pooled, _ = _run(inputs, trace=False)
    return pooled
